# revision 33
# baseline (speedup 1.0000x reference)
"""Single-head causal attention (B=8, T=2048, C=768, H=64) on 8 TRN2 NeuronCores.

Sharding: data-parallel over the batch dim — one batch element per core.

Per-core algorithm (bf16 matmul operands, fp32 PSUM accumulation):
  - x fed as 12 host-prearranged CONTIGUOUS pieces in need-order on the
    sync HWDGE ring only (two rings round-robin the same 16 SDMA engines
    and halve each other): six single-chunk [128, 512] pieces so qk
    group 0 accumulates per-chunk as pieces land, then six [128, 3, 512]
    pieces. ScalarE's queue stays free of DMA issues for the exp stream.
  - warmup matmuls interleave with the piece-gated qk chunks to keep the
    PE HAM activity monitor busy (else it re-throttles to half clock).
  - qkT [128, T]: rows 0:64 = q^T, 64:128 = k^T (fused [Wq | Wk] weights).
  - qk2 [128, T]: rows 0:64 = k^T, rows 64:128 = q^T — built per 512-col
    group by ONE PE matmul against a constant half-swap permutation
    (SBUF->SBUF DMA shifts take 4-5us while the x stream owns the SDMA
    engines; the swap matmul takes ~0.2us).
  - scores are ROW-SPLIT on the PE: the K=64 contraction only needs half
    the array, so each "unit" runs two independent score matmuls
    concurrently — tile A in array rows 0:64 (lhsT = qk2[0:64] k-shift,
    rhs = qkT[0:64] q), tile B in rows 64:128 (lhsT = qkT[64:128]
    original k, rhs = qk2[64:128] q-dup) — ~2x score throughput. The two
    tiles write DIFFERENT PSUM banks of one [128, 1024] tile (B always
    at col 512): concurrent row-tile drains into one bank hang the HW.
    Tile A extends short items to 512 cols (discarded scores) so the
    packed exp never reads unwritten PSUM.
  - one exp activation per unit over the packed span (ScalarE),
    amortizing the ~352-cycle ACTIVATE overhead; diagonal 128-blocks are
    masked post-exp with an upper-triangular multiply on VectorE.
  - AV: out^T [65, 512] += [v_j | 1].T @ expS^T_j per half (row 64
    accumulates softmax denominators); halves evacuate (PSUM -> SBUF
    bf16, VectorE) as soon as their last j-chunk lands, then DMA out.
  - v path: vT [64, T] = Wv^T @ x^T, PE-transposed per 128-token chunk
    into v80 [128, 16, 80] natural-layout tiles ([v_j | 1]).
  - remaining QKV groups, v matmuls and transposes are injected into the
    unit stream sized to the per-unit PE slack under the exp stream.
  - output is oT [65, T] bf16 (unnormalized + denominators); the host
    does out = (oT[:64] / oT[64:65]).T in fp32.

No max-subtraction in softmax: scores * C**-0.5 are bounded (|s| < ~3), exp is
safe in fp32, and the result is mathematically identical to jax.nn.softmax.
"""

import ml_dtypes
import numpy as np

import concourse.bass as bass
import concourse.tile as tile
from concourse import bacc, mybir
from concourse.bass import ds, ts
from concourse.masks import make_identity, make_upper_triangular

B, T, C, H = 8, 2048, 768, 64
P = 128
NCH = C // P          # 6 contraction chunks for QKV
GW = 1024             # attention output column-group width
NG = T // GW          # 2 groups
NT = T // P           # 16 t-chunks
JPG = GW // P         # 8 j-chunks per group
VP = 80               # vT partition rows (64 v + pad to 16x for tile pools)
SCALE = float(C) ** -0.5
N_WARMUP = 5
ROW_SPLIT = True

# x pieces, all on the sync HWDGE ring in need-order: six single-chunk
# pieces (qk group 0 accumulates per-chunk as they land), then six
# 3-chunk pieces. One ring keeps all 16 SDMA engines on one stream (two
# rings round-robin and halve each other) and keeps ScalarE's queue free
# of DMA issues for the exp stream.
PIECES_A = [(c, 0) for c in range(6)]              # (c0, t0), 1 chunk
PIECES_B = [(c0, t0) for t0 in (512, 1024, 1536) for c0 in (0, 3)]

F32 = mybir.dt.float32
BF16 = mybir.dt.bfloat16
EXP = mybir.ActivationFunctionType.Exp


def _units():
    """Stream of score units per attention group.

    Each item is (jj, c0, n): key chunk jj, group-relative query cols
    [c0, c0+n). Units pair two items; item 0 -> PE row-tile A (k-shifted
    weights), item 1 -> row-tile B (original k, q-dup moving operand).
    s0 items (which carry the diagonal block when jj is diagonal) come
    first so early units only need the first shift pieces.
    """
    units = []
    for g in range(NG):
        items_s = []   # (jj, c0, n) first-seg items
        items_l = []   # second-seg items
        for jj in range(JPG * g + JPG):
            istart = jj * P - g * GW  # group-relative start (>=0 on diag)
            istart = max(istart, 0)
            if istart < 512:
                items_s.append((jj, istart, 512 - istart))
                items_l.append((jj, 512, 512))
            else:
                items_s.append((jj, istart, GW - istart))
        its = items_s + items_l
        for k in range(0, len(its), 2):
            units.append((g, its[k : k + 2]))
    return units


def _emit(tc: tile.TileContext, ctx, xp, wqk, wv, oT):
    nc = tc.nc

    consts = ctx.enter_context(tc.tile_pool(name="consts", bufs=1))
    xpool = ctx.enter_context(tc.tile_pool(name="x", bufs=1))
    qpool = ctx.enter_context(tc.tile_pool(name="qkv", bufs=1))

    # ---- t=0: input DMAs (both HWDGE engines), exp-table preload ----
    xT_sb = xpool.tile([P, NCH, T], BF16)

    # preload the exp table set first (scalar engine, overlaps everything)
    dummy = consts.tile([P, 1], F32)
    nc.vector.memset(dummy[:], 0.0)
    nc.scalar.activation(dummy[:], dummy[:], EXP)
    # weights lead their rings: they gate the very first QKV LDWEIGHTS
    w_qk = consts.tile([P, NCH, P], BF16)
    nc.sync.dma_start(w_qk[:], wqk)
    w_v = consts.tile([P, NCH, H], BF16)
    nc.scalar.dma_start(w_v[:], wv)
    # contiguous pieces, need-order on the sync ring
    xpA, xpB = xp
    for k, (c0, t0) in enumerate(PIECES_A):
        nc.sync.dma_start(xT_sb[:, c0, ds(t0, 512)], xpA[k])
    for k, (c0, t0) in enumerate(PIECES_B):
        nc.sync.dma_start(xT_sb[:, c0 : c0 + 3, ds(t0, 512)], xpB[k])

    # warmup tile for dummy matmuls
    dum = qpool.tile([P, 512], BF16)
    nc.vector.memset(dum[:], 0.0)

    ident = consts.tile([H, H], BF16)
    make_identity(nc, ident[:])
    tri = consts.tile([P, P], BF16)
    make_upper_triangular(nc, tri[:], val=1.0, diag=True)
    # half-swap permutation: swap.T @ x exchanges partition halves
    swap = consts.tile([P, P], BF16)
    nc.vector.memset(swap[:], 0.0)
    make_identity(nc, swap[H:P, 0:H])
    make_identity(nc, swap[0:H, H:P])

    qkT = qpool.tile([P, T], BF16)
    qk2 = qpool.tile([P, T], BF16)   # rows 0:64 k-shift, rows 64:128 q-dup
    vT80 = qpool.tile([VP, T], BF16)
    v80 = qpool.tile([P, NT, VP], BF16)
    # ones column for the AV denominator rows
    nc.vector.memset(v80[:, :, H : H + 1], 1.0)

    qk_ps = {}
    v_ps = {}

    def qk_mm(g, cs, pool, tag, warm=None):
        if g not in qk_ps:
            qk_ps[g] = pool.tile([P, 512], F32, tag=tag, name=f"qk_{g}")
        ps = qk_ps[g]
        for c in cs:
            nc.tensor.matmul(
                ps[:],
                w_qk[:, c, :],
                xT_sb[:, c, ts(g, 512)],
                start=(c == 0),
                stop=(c == NCH - 1),
            )
            if warm is not None and c < NCH - 1:
                # keep the PE HAM activity monitor busy between the
                # piece-gated chunk matmuls
                warm(1)

    def qk_fin(g, pool, tag):
        nc.vector.tensor_copy(qkT[:, ts(g, 512)], qk_ps[g][:])
        # build qk2 for these 512 cols — k^T down to rows 0:64 (tile-A
        # weights), q^T up to rows 64:128 (tile-B moving operand) — via
        # a PE half-swap matmul (~0.2us; SBUF->SBUF DMA shifts take
        # 4-5us while the x stream owns the SDMA engines).
        sw = pool.tile([P, 512], F32, tag=tag, name=f"sw_{g}")
        nc.tensor.matmul(sw[:], swap[:], qkT[:, ts(g, 512)],
                         start=True, stop=True)
        nc.vector.tensor_copy(qk2[:, ts(g, 512)], sw[:])

    def v_mm(g, cs, pool, tag):
        if g not in v_ps:
            v_ps[g] = pool.tile([P, 512], F32, tag=tag, name=f"v_{g}")
        ps = v_ps[g][0:H, :]
        for c in cs:
            nc.tensor.matmul(
                ps,
                w_v[:, c, :],
                xT_sb[:, c, ts(g, 512)],
                start=(c == 0),
                stop=(c == NCH - 1),
            )

    def v_fin(g):
        nc.vector.tensor_copy(vT80[0:H, ts(g, 512)], v_ps[g][0:H, :])

    # ---- attention stream pools (created first so sp coexists with pa) ----
    sp = ctx.enter_context(tc.tile_pool(name="spsum", bufs=2, space="PSUM"))
    pb = ctx.enter_context(tc.tile_pool(name="probs", bufs=6))
    fin = ctx.enter_context(tc.tile_pool(name="fin", bufs=3))

    units = _units()

    def emit_unit(g, items, ext_lim=GW):
        """Row-split scores for up to two items packed in one PSUM tile,
        one exp over the packed span, diag masks. Returns
        [(prb, poff, jj, c0, n), ...] for the AV stage."""
        sps = sp.tile([P, GW], F32, tag="s")
        offs, spans, off = [], [], 0
        for role, (jj, c0, n) in enumerate(items):
            ne = n
            if ROW_SPLIT:
                # tile B always starts at col 512: its own PSUM bank, so
                # the two concurrent row-tile drains never share a bank.
                # Tile A extends to 512 cols (extra scores are discarded)
                # where the group's qkT columns allow, so the packed
                # activation below never reads unwritten PSUM.
                poff = 0 if role == 0 else 512
                if role == 0 and len(items) > 1:
                    ne = min(512, ext_lim - c0)
            else:
                poff = off
            offs.append(poff)
            spans.append((poff, ne))
            off = poff + n
            if role == 0 or not ROW_SPLIT:
                lhsT = qk2[0:H, ts(jj, P)]        # k-shift, rows 0:64
                rhs = qkT[0:H, ds(g * GW + c0, ne)]
            else:
                lhsT = qkT[H:P, ts(jj, P)]        # original k, rows 64:128
                rhs = qk2[H:P, ds(g * GW + c0, ne)]
            nc.tensor.matmul(sps[:, poff : poff + ne], lhsT, rhs,
                             start=True, stop=True)
        tot = off
        prb = pb.tile([P, GW], BF16, tag="p")
        if ROW_SPLIT and len(items) > 1 and spans[0][1] < 512:
            # unextendable hole before col 512: activate per item
            for poff, ne in spans:
                nc.scalar.activation(prb[:, poff : poff + ne],
                                     sps[:, poff : poff + ne], EXP,
                                     scale=SCALE)
        else:
            nc.scalar.activation(prb[:, :tot], sps[:, :tot], EXP,
                                 scale=SCALE)
        out = []
        for poff, (jj, c0, n) in zip(offs, items):
            if jj * P >= g * GW and c0 == jj * P - g * GW:
                # leading 128 cols are the diagonal block: upper-tri mask
                nc.vector.tensor_mul(out=prb[:, poff : poff + P],
                                     in0=prb[:, poff : poff + P], in1=tri[:])
            out.append((prb, poff, jj, c0, n))
        return out

    # ---- head: warmups + qk 0/1 interleaved with the first score units ----
    prb_queue = []
    wcount = [0]
    with tc.tile_pool(name="papsum", bufs=2, space="PSUM") as pa:
        def warm(k):
            for _ in range(k):
                dps = pa.tile([P, 512], F32, tag="qk", name=f"warm_{wcount[0]}")
                wcount[0] += 1
                nc.tensor.matmul(dps[:], dum[:, 0:P], dum[:],
                                 start=True, stop=True)

        warm(N_WARMUP)
        qk_mm(0, range(NCH), pa, "qk", warm=warm)
        qk_fin(0, pa, "qk")
        warm(2)
        prb_queue.append(emit_unit(*units[0], ext_lim=512))
        prb_queue.append(emit_unit(*units[1], ext_lim=512))
        qk_mm(1, range(NCH), pa, "qk", warm=warm)
        qk_fin(1, pa, "qk")
        prb_queue.append(emit_unit(*units[2]))

    # pool B: v matmuls + late qk (shared bank) + PE-transpose staging
    pq = ctx.enter_context(tc.tile_pool(name="pbpsum", bufs=1, space="PSUM"))
    op = ctx.enter_context(tc.tile_pool(name="opsum", bufs=2, space="PSUM"))

    def pe_transpose(t):
        pt = pq.tile([P, H], BF16, tag="vt", name=f"vt_{t}")
        nc.tensor.transpose(pt[:], vT80[0:H, ts(t, P)], ident[:])
        nc.vector.tensor_copy(v80[:, t, 0:H], pt[:])

    inject = {
        # stagger v/transpose work just-in-time (unit k's AV only needs
        # its own j-chunks' v80 tiles; group-1 scores need fins 2,3), as
        # early as x-piece arrival allows, so the injected work never
        # sits in the PE FIFO when the exp stream is waiting for scores
        -1: lambda: (v_mm(0, range(NCH), pq, "v"), v_fin(0),
                     pe_transpose(0), pe_transpose(1)),
        0: lambda: (pe_transpose(2), pe_transpose(3),
                    qk_mm(2, range(NCH), pq, "v"), qk_fin(2, pq, "v")),
        1: lambda: (v_mm(1, range(NCH), pq, "v"), v_fin(1),
                    pe_transpose(4), pe_transpose(5)),
        2: lambda: (pe_transpose(6), pe_transpose(7)),
        4: lambda: qk_mm(3, range(3), pq, "v"),
        5: lambda: (qk_mm(3, range(3, 6), pq, "v"), qk_fin(3, pq, "v")),
        6: lambda: v_mm(2, range(3), pq, "v"),
        7: lambda: (v_mm(2, range(3, 6), pq, "v"), v_fin(2)),
        8: lambda: (pe_transpose(8), pe_transpose(9),
                    v_mm(3, range(3), pq, "v")),
        9: lambda: (v_mm(3, range(3, 6), pq, "v"), v_fin(3),
                    pe_transpose(10), pe_transpose(11)),
        10: lambda: (pe_transpose(12), pe_transpose(13)),
        11: lambda: (pe_transpose(14), pe_transpose(15)),
    }

    def emit_evac(g, hh, oph, last):
        osb = fin.tile([H + 1, 512], BF16, tag="osb", name=f"osb_{g}_{hh}")
        nc.vector.tensor_copy(osb[:], oph[:])
        eng = nc.sync if last else nc.gpsimd
        eng.dma_start(oT[:, ds(g * GW + hh * 512, 512)], osb[:])

    # AV bookkeeping: per (g, half): item count, seen count
    half_tot = {}
    for g, items in units:
        for jj, c0, n in items:
            half_tot[(g, c0 // 512)] = half_tot.get((g, c0 // 512), 0) + 1
    half_seen = {}
    ops_by_gh = {}
    n_units = len(units)

    LOOKAHEAD = 4
    ei = len(prb_queue)
    inject[-1]()
    for idx, (g, items) in enumerate(units):
        unit_prbs = prb_queue.pop(0)
        for prb, poff, jj, c0, n in unit_prbs:
            hh = c0 // 512
            key = (g, hh)
            if key not in ops_by_gh:
                ops_by_gh[key] = op.tile(
                    [H + 1, 512], F32, tag="o", name=f"ops_{g}_{hh}"
                )
            oph = ops_by_gh[key]
            seen = half_seen.get(key, 0) + 1
            half_seen[key] = seen
            is_last = seen == half_tot[key]
            nc.tensor.matmul(
                oph[:, c0 - hh * 512 : c0 - hh * 512 + n],
                v80[:, jj, 0 : H + 1],
                prb[:, poff : poff + n],
                start=(seen == 1),
                stop=is_last,
            )
            if is_last:
                emit_evac(g, hh, oph, last=(idx == n_units - 1))

        if idx in inject:
            inject[idx]()
        while len(prb_queue) < LOOKAHEAD and ei < n_units:
            prb_queue.append(emit_unit(*units[ei]))
            ei += 1


def build():
    from contextlib import ExitStack

    nc = bacc.Bacc("TRN2", target_bir_lowering=False, debug=False, num_devices=B)
    xpA = nc.dram_tensor("xpA", [len(PIECES_A), P, 512], BF16,
                         kind="ExternalInput").ap()
    xpB = nc.dram_tensor("xpB", [len(PIECES_B), P, 3 * 512], BF16,
                         kind="ExternalInput").ap()
    xp = (xpA, xpB)
    wqk = nc.dram_tensor("wqk", [P, NCH, P], BF16, kind="ExternalInput").ap()
    wv = nc.dram_tensor("wv", [P, NCH, H], BF16, kind="ExternalInput").ap()
    oT = nc.dram_tensor("oT", [H + 1, T], BF16, kind="ExternalOutput").ap()
    with tile.TileContext(nc) as tc, ExitStack() as ctx:
        _emit(tc, ctx, xp, wqk, wv, oT)
    nc.compile()
    return nc


_NC = None


def _get_nc():
    global _NC
    if _NC is None:
        _NC = build()
    return _NC


def make_in_maps(x, Wk, Wq, Wv):
    bf = ml_dtypes.bfloat16
    wqk = np.concatenate([Wq, Wk], axis=1).astype(bf)          # [C, 128]
    wqk = np.ascontiguousarray(wqk.reshape(NCH, P, P).transpose(1, 0, 2))
    wvp = np.asarray(Wv).astype(bf)
    wvp = np.ascontiguousarray(wvp.reshape(NCH, P, H).transpose(1, 0, 2))
    maps = []
    for b in range(B):
        xT = np.asarray(x[b]).T.astype(bf)                     # [C, T]
        xr = xT.reshape(NCH, P, T)
        xpa = np.stack([
            np.ascontiguousarray(xr[c0, :, t0 : t0 + 512])
            for (c0, t0) in PIECES_A
        ])
        xpb = np.stack([
            np.ascontiguousarray(
                xr[c0 : c0 + 3, :, t0 : t0 + 512].transpose(1, 0, 2)
            ).reshape(P, 3 * 512)
            for (c0, t0) in PIECES_B
        ])
        maps.append({"xpA": xpa, "xpB": xpb, "wqk": wqk, "wv": wvp})
    return maps


def finalize_host(oT):
    """oT [65, T] bf16 -> normalized [T, H] fp32 output."""
    oT = np.asarray(oT, dtype=np.float32)
    return np.ascontiguousarray((oT[:H] / oT[H : H + 1]).T, dtype=np.float32)


def kernel(x, Wk, Wq, Wv):
    from concourse.bass_utils import run_bass_kernel_spmd

    nc = _get_nc()
    in_maps = make_in_maps(x, Wk, Wq, Wv)
    r = run_bass_kernel_spmd(nc, in_maps, core_ids=list(range(B)))
    out = np.stack([finalize_host(r.results[b]["oT"]) for b in range(B)])
    return np.ascontiguousarray(out, dtype=np.float32)


# revision 35
# speedup vs baseline: 1.0302x; 1.0302x over previous
"""Single-head causal attention (B=8, T=2048, C=768, H=64) on 8 TRN2 NeuronCores.

Sharding: data-parallel over the batch dim — one batch element per core.

Per-core algorithm (bf16 matmul operands, fp32 PSUM accumulation):
  - x fed as 12 host-prearranged CONTIGUOUS pieces in need-order on the
    sync HWDGE ring only (two rings round-robin the same 16 SDMA engines
    and halve each other): six single-chunk [128, 512] pieces so qk
    group 0 accumulates per-chunk as pieces land, then six [128, 3, 512]
    pieces. ScalarE's queue stays free of DMA issues for the exp stream.
  - warmup matmuls interleave with the piece-gated qk chunks to keep the
    PE HAM activity monitor busy (else it re-throttles to half clock).
  - qkT [128, T]: rows 0:64 = q^T, 64:128 = k^T (fused [Wq | Wk] weights).
  - qk2 [128, T]: rows 0:64 = k^T, rows 64:128 = q^T — built per 512-col
    group by ONE PE matmul against a constant half-swap permutation
    (SBUF->SBUF DMA shifts take 4-5us while the x stream owns the SDMA
    engines; the swap matmul takes ~0.2us).
  - scores are ROW-SPLIT on the PE: the K=64 contraction only needs half
    the array, so each "unit" runs two independent score matmuls
    concurrently — tile A in array rows 0:64 (lhsT = qk2[0:64] k-shift,
    rhs = qkT[0:64] q), tile B in rows 64:128 (lhsT = qkT[64:128]
    original k, rhs = qk2[64:128] q-dup) — ~2x score throughput. The two
    tiles write DIFFERENT PSUM banks of one [128, 1024] tile (B always
    at col 512): concurrent row-tile drains into one bank hang the HW.
    Tile A extends short items to 512 cols (discarded scores) so the
    packed exp never reads unwritten PSUM.
  - one exp activation per unit over the packed span (ScalarE),
    amortizing the ~352-cycle ACTIVATE overhead; diagonal 128-blocks are
    masked post-exp with an upper-triangular multiply on VectorE.
  - AV: out^T [65, 512] += [v_j | 1].T @ expS^T_j per half (row 64
    accumulates softmax denominators); halves evacuate (PSUM -> SBUF
    bf16, VectorE) as soon as their last j-chunk lands, then DMA out.
  - v path: vT [64, T] = Wv^T @ x^T, PE-transposed per 128-token chunk
    into v80 [128, 16, 80] natural-layout tiles ([v_j | 1]).
  - remaining QKV groups, v matmuls and transposes are injected into the
    unit stream sized to the per-unit PE slack under the exp stream.
  - output is oT [65, T] bf16 (unnormalized + denominators); the host
    does out = (oT[:64] / oT[64:65]).T in fp32.

No max-subtraction in softmax: scores * C**-0.5 are bounded (|s| < ~3), exp is
safe in fp32, and the result is mathematically identical to jax.nn.softmax.
"""

import ml_dtypes
import numpy as np

import concourse.bass as bass
import concourse.tile as tile
from concourse import bacc, mybir
from concourse.bass import ds, ts
from concourse.masks import make_identity, make_upper_triangular

B, T, C, H = 8, 2048, 768, 64
P = 128
NCH = C // P          # 6 contraction chunks for QKV
GW = 1024             # attention output column-group width
NG = T // GW          # 2 groups
NT = T // P           # 16 t-chunks
JPG = GW // P         # 8 j-chunks per group
VP = 80               # vT partition rows (64 v + pad to 16x for tile pools)
SCALE = float(C) ** -0.5
N_WARMUP = 5
ROW_SPLIT = True

# x pieces, all on the sync HWDGE ring in need-order: six single-chunk
# pieces (qk group 0 accumulates per-chunk as they land), then six
# 3-chunk pieces. One ring keeps all 16 SDMA engines on one stream (two
# rings round-robin and halve each other) and keeps ScalarE's queue free
# of DMA issues for the exp stream.
PIECES_A = [(c, 0) for c in range(6)]              # (c0, t0), 1 chunk
PIECES_B = [(c0, t0) for t0 in (512, 1024, 1536) for c0 in (0, 3)]

F32 = mybir.dt.float32
BF16 = mybir.dt.bfloat16
EXP = mybir.ActivationFunctionType.Exp


def _units():
    """Stream of score units per attention group.

    Each item is (jj, c0, n): key chunk jj, group-relative query cols
    [c0, c0+n). Units pair two items; item 0 -> PE row-tile A (k-shifted
    weights), item 1 -> row-tile B (original k, q-dup moving operand).
    s0 items (which carry the diagonal block when jj is diagonal) come
    first so early units only need the first shift pieces.
    """
    units = []
    for g in range(NG):
        items_s = []   # (jj, c0, n) first-seg items
        items_l = []   # second-seg items
        for jj in range(JPG * g + JPG):
            istart = jj * P - g * GW  # group-relative start (>=0 on diag)
            istart = max(istart, 0)
            if istart < 512:
                items_s.append((jj, istart, 512 - istart))
                items_l.append((jj, 512, 512))
            else:
                items_s.append((jj, istart, GW - istart))
        its = items_s + items_l
        for k in range(0, len(its), 2):
            units.append((g, its[k : k + 2]))
    return units


def _emit(tc: tile.TileContext, ctx, xp, wqk, wv, oT):
    nc = tc.nc

    consts = ctx.enter_context(tc.tile_pool(name="consts", bufs=1))
    xpool = ctx.enter_context(tc.tile_pool(name="x", bufs=1))
    qpool = ctx.enter_context(tc.tile_pool(name="qkv", bufs=1))

    # ---- t=0: input DMAs (both HWDGE engines), exp-table preload ----
    xT_sb = xpool.tile([P, NCH, T], BF16)

    # preload the exp table set first (scalar engine, overlaps everything)
    dummy = consts.tile([P, 1], F32)
    nc.vector.memset(dummy[:], 0.0)
    nc.scalar.activation(dummy[:], dummy[:], EXP)
    # weights on the scalar ring so the sync ring is pure x and the
    # first qk chunk piece lands earlier (rings share the SDMA engines,
    # but this removes wqk from in FRONT of the first x piece)
    w_qk = consts.tile([P, NCH, P], BF16)
    nc.scalar.dma_start(w_qk[:], wqk)
    w_v = consts.tile([P, NCH, H], BF16)
    nc.scalar.dma_start(w_v[:], wv)
    # contiguous pieces, need-order on the sync ring
    xpA, xpB = xp
    for k, (c0, t0) in enumerate(PIECES_A):
        nc.sync.dma_start(xT_sb[:, c0, ds(t0, 512)], xpA[k])
    for k, (c0, t0) in enumerate(PIECES_B):
        nc.sync.dma_start(xT_sb[:, c0 : c0 + 3, ds(t0, 512)], xpB[k])

    # warmup tile for dummy matmuls
    dum = qpool.tile([P, 512], BF16)
    nc.vector.memset(dum[:], 0.0)

    ident = consts.tile([H, H], BF16)
    make_identity(nc, ident[:])
    tri = consts.tile([P, P], BF16)
    make_upper_triangular(nc, tri[:], val=1.0, diag=True)
    # half-swap permutation: swap.T @ x exchanges partition halves
    swap = consts.tile([P, P], BF16)
    nc.vector.memset(swap[:], 0.0)
    make_identity(nc, swap[H:P, 0:H])
    make_identity(nc, swap[0:H, H:P])

    qkT = qpool.tile([P, T], BF16)
    qk2 = qpool.tile([P, T], BF16)   # rows 0:64 k-shift, rows 64:128 q-dup
    vT80 = qpool.tile([VP, T], BF16)
    v80 = qpool.tile([P, NT, VP], BF16)
    # ones column for the AV denominator rows
    nc.vector.memset(v80[:, :, H : H + 1], 1.0)

    qk_ps = {}
    v_ps = {}

    def qk_mm(g, cs, pool, tag, warm=None):
        if g not in qk_ps:
            qk_ps[g] = pool.tile([P, 512], F32, tag=tag, name=f"qk_{g}")
        ps = qk_ps[g]
        for c in cs:
            nc.tensor.matmul(
                ps[:],
                w_qk[:, c, :],
                xT_sb[:, c, ts(g, 512)],
                start=(c == 0),
                stop=(c == NCH - 1),
            )
            if warm is not None and c < NCH - 1:
                # keep the PE HAM activity monitor busy between the
                # piece-gated chunk matmuls
                warm(1)

    def qk_fin(g, pool, tag):
        nc.vector.tensor_copy(qkT[:, ts(g, 512)], qk_ps[g][:])
        # build qk2 for these 512 cols — k^T down to rows 0:64 (tile-A
        # weights), q^T up to rows 64:128 (tile-B moving operand) — via
        # a PE half-swap matmul (~0.2us; SBUF->SBUF DMA shifts take
        # 4-5us while the x stream owns the SDMA engines).
        sw = pool.tile([P, 512], F32, tag=tag, name=f"sw_{g}")
        nc.tensor.matmul(sw[:], swap[:], qkT[:, ts(g, 512)],
                         start=True, stop=True)
        nc.vector.tensor_copy(qk2[:, ts(g, 512)], sw[:])

    def v_mm(g, cs, pool, tag):
        if g not in v_ps:
            v_ps[g] = pool.tile([P, 512], F32, tag=tag, name=f"v_{g}")
        ps = v_ps[g][0:H, :]
        for c in cs:
            nc.tensor.matmul(
                ps,
                w_v[:, c, :],
                xT_sb[:, c, ts(g, 512)],
                start=(c == 0),
                stop=(c == NCH - 1),
            )

    def v_fin(g):
        nc.vector.tensor_copy(vT80[0:H, ts(g, 512)], v_ps[g][0:H, :])

    # ---- attention stream pools (created first so sp coexists with pa) ----
    sp = ctx.enter_context(tc.tile_pool(name="spsum", bufs=2, space="PSUM"))
    pb = ctx.enter_context(tc.tile_pool(name="probs", bufs=6))
    fin = ctx.enter_context(tc.tile_pool(name="fin", bufs=3))

    units = _units()

    def emit_unit(g, items, ext_lim=GW):
        """Row-split scores for up to two items packed in one PSUM tile,
        one exp over the packed span, diag masks. Returns
        [(prb, poff, jj, c0, n), ...] for the AV stage."""
        sps = sp.tile([P, GW], F32, tag="s")
        offs, spans, off = [], [], 0
        for role, (jj, c0, n) in enumerate(items):
            ne = n
            if ROW_SPLIT:
                # tile B always starts at col 512: its own PSUM bank, so
                # the two concurrent row-tile drains never share a bank.
                # Tile A extends to 512 cols (extra scores are discarded)
                # where the group's qkT columns allow, so the packed
                # activation below never reads unwritten PSUM.
                poff = 0 if role == 0 else 512
                if role == 0 and len(items) > 1:
                    ne = min(512, ext_lim - c0)
            else:
                poff = off
            offs.append(poff)
            spans.append((poff, ne))
            off = poff + n
            if role == 0 or not ROW_SPLIT:
                lhsT = qk2[0:H, ts(jj, P)]        # k-shift, rows 0:64
                rhs = qkT[0:H, ds(g * GW + c0, ne)]
            else:
                lhsT = qkT[H:P, ts(jj, P)]        # original k, rows 64:128
                rhs = qk2[H:P, ds(g * GW + c0, ne)]
            nc.tensor.matmul(sps[:, poff : poff + ne], lhsT, rhs,
                             start=True, stop=True)
        tot = off
        prb = pb.tile([P, GW], BF16, tag="p")
        if ROW_SPLIT and len(items) > 1 and spans[0][1] < 512:
            # unextendable hole before col 512: activate per item
            for poff, ne in spans:
                nc.scalar.activation(prb[:, poff : poff + ne],
                                     sps[:, poff : poff + ne], EXP,
                                     scale=SCALE)
        else:
            nc.scalar.activation(prb[:, :tot], sps[:, :tot], EXP,
                                 scale=SCALE)
        out = []
        for poff, (jj, c0, n) in zip(offs, items):
            if jj * P >= g * GW and c0 == jj * P - g * GW:
                # leading 128 cols are the diagonal block: upper-tri mask
                nc.vector.tensor_mul(out=prb[:, poff : poff + P],
                                     in0=prb[:, poff : poff + P], in1=tri[:])
            out.append((prb, poff, jj, c0, n))
        return out

    # ---- head: warmups + qk 0/1 interleaved with the first score units ----
    prb_queue = []
    wcount = [0]
    with tc.tile_pool(name="papsum", bufs=2, space="PSUM") as pa:
        def warm(k):
            for _ in range(k):
                dps = pa.tile([P, 512], F32, tag="qk", name=f"warm_{wcount[0]}")
                wcount[0] += 1
                nc.tensor.matmul(dps[:], dum[:, 0:P], dum[:],
                                 start=True, stop=True)

        warm(N_WARMUP)
        qk_mm(0, range(NCH), pa, "qk", warm=warm)
        qk_fin(0, pa, "qk")
        warm(2)
        prb_queue.append(emit_unit(*units[0], ext_lim=512))
        prb_queue.append(emit_unit(*units[1], ext_lim=512))
        qk_mm(1, range(NCH), pa, "qk", warm=warm)
        qk_fin(1, pa, "qk")
        prb_queue.append(emit_unit(*units[2]))

    # pool B: v matmuls + late qk (shared bank) + PE-transpose staging
    pq = ctx.enter_context(tc.tile_pool(name="pbpsum", bufs=1, space="PSUM"))
    op = ctx.enter_context(tc.tile_pool(name="opsum", bufs=2, space="PSUM"))

    def pe_transpose(t):
        pt = pq.tile([P, H], BF16, tag="vt", name=f"vt_{t}")
        nc.tensor.transpose(pt[:], vT80[0:H, ts(t, P)], ident[:])
        nc.vector.tensor_copy(v80[:, t, 0:H], pt[:])

    inject = {
        # stagger v/transpose work just-in-time (unit k's AV only needs
        # its own j-chunks' v80 tiles; group-1 scores need fins 2,3), as
        # early as x-piece arrival allows, so the injected work never
        # sits in the PE FIFO when the exp stream is waiting for scores
        -1: lambda: (v_mm(0, range(NCH), pq, "v"), v_fin(0),
                     pe_transpose(0), pe_transpose(1)),
        0: lambda: (pe_transpose(2), pe_transpose(3),
                    v_mm(1, range(3), pq, "v")),
        1: lambda: (v_mm(1, range(3, 6), pq, "v"), v_fin(1),
                    pe_transpose(4), pe_transpose(5)),
        2: lambda: (pe_transpose(6), pe_transpose(7),
                    qk_mm(2, range(3), pq, "v")),
        3: lambda: (qk_mm(2, range(3, 6), pq, "v"), qk_fin(2, pq, "v")),
        4: lambda: qk_mm(3, range(3), pq, "v"),
        5: lambda: (qk_mm(3, range(3, 6), pq, "v"), qk_fin(3, pq, "v")),
        6: lambda: v_mm(2, range(3), pq, "v"),
        7: lambda: (v_mm(2, range(3, 6), pq, "v"), v_fin(2)),
        8: lambda: (pe_transpose(8), pe_transpose(9),
                    v_mm(3, range(3), pq, "v")),
        9: lambda: (v_mm(3, range(3, 6), pq, "v"), v_fin(3),
                    pe_transpose(10), pe_transpose(11)),
        10: lambda: (pe_transpose(12), pe_transpose(13)),
        11: lambda: (pe_transpose(14), pe_transpose(15)),
    }

    def emit_evac(g, hh, oph, last):
        osb = fin.tile([H + 1, 512], BF16, tag="osb", name=f"osb_{g}_{hh}")
        nc.vector.tensor_copy(osb[:], oph[:])
        eng = nc.sync if last else nc.gpsimd
        eng.dma_start(oT[:, ds(g * GW + hh * 512, 512)], osb[:])

    # AV bookkeeping: per (g, half): item count, seen count
    half_tot = {}
    for g, items in units:
        for jj, c0, n in items:
            half_tot[(g, c0 // 512)] = half_tot.get((g, c0 // 512), 0) + 1
    half_seen = {}
    ops_by_gh = {}
    n_units = len(units)

    LOOKAHEAD = 3
    ei = len(prb_queue)
    inject[-1]()
    for idx, (g, items) in enumerate(units):
        unit_prbs = prb_queue.pop(0)
        for prb, poff, jj, c0, n in unit_prbs:
            hh = c0 // 512
            key = (g, hh)
            if key not in ops_by_gh:
                ops_by_gh[key] = op.tile(
                    [H + 1, 512], F32, tag="o", name=f"ops_{g}_{hh}"
                )
            oph = ops_by_gh[key]
            seen = half_seen.get(key, 0) + 1
            half_seen[key] = seen
            is_last = seen == half_tot[key]
            nc.tensor.matmul(
                oph[:, c0 - hh * 512 : c0 - hh * 512 + n],
                v80[:, jj, 0 : H + 1],
                prb[:, poff : poff + n],
                start=(seen == 1),
                stop=is_last,
            )
            if is_last:
                emit_evac(g, hh, oph, last=(idx == n_units - 1))

        if idx in inject:
            inject[idx]()
        while len(prb_queue) < LOOKAHEAD and ei < n_units:
            prb_queue.append(emit_unit(*units[ei]))
            ei += 1


def build():
    from contextlib import ExitStack

    nc = bacc.Bacc("TRN2", target_bir_lowering=False, debug=False, num_devices=B)
    xpA = nc.dram_tensor("xpA", [len(PIECES_A), P, 512], BF16,
                         kind="ExternalInput").ap()
    xpB = nc.dram_tensor("xpB", [len(PIECES_B), P, 3 * 512], BF16,
                         kind="ExternalInput").ap()
    xp = (xpA, xpB)
    wqk = nc.dram_tensor("wqk", [P, NCH, P], BF16, kind="ExternalInput").ap()
    wv = nc.dram_tensor("wv", [P, NCH, H], BF16, kind="ExternalInput").ap()
    oT = nc.dram_tensor("oT", [H + 1, T], BF16, kind="ExternalOutput").ap()
    with tile.TileContext(nc) as tc, ExitStack() as ctx:
        _emit(tc, ctx, xp, wqk, wv, oT)
    nc.compile()
    return nc


_NC = None


def _get_nc():
    global _NC
    if _NC is None:
        _NC = build()
    return _NC


def make_in_maps(x, Wk, Wq, Wv):
    bf = ml_dtypes.bfloat16
    wqk = np.concatenate([Wq, Wk], axis=1).astype(bf)          # [C, 128]
    wqk = np.ascontiguousarray(wqk.reshape(NCH, P, P).transpose(1, 0, 2))
    wvp = np.asarray(Wv).astype(bf)
    wvp = np.ascontiguousarray(wvp.reshape(NCH, P, H).transpose(1, 0, 2))
    maps = []
    for b in range(B):
        xT = np.asarray(x[b]).T.astype(bf)                     # [C, T]
        xr = xT.reshape(NCH, P, T)
        xpa = np.stack([
            np.ascontiguousarray(xr[c0, :, t0 : t0 + 512])
            for (c0, t0) in PIECES_A
        ])
        xpb = np.stack([
            np.ascontiguousarray(
                xr[c0 : c0 + 3, :, t0 : t0 + 512].transpose(1, 0, 2)
            ).reshape(P, 3 * 512)
            for (c0, t0) in PIECES_B
        ])
        maps.append({"xpA": xpa, "xpB": xpb, "wqk": wqk, "wv": wvp})
    return maps


def finalize_host(oT):
    """oT [65, T] bf16 -> normalized [T, H] fp32 output."""
    oT = np.asarray(oT, dtype=np.float32)
    return np.ascontiguousarray((oT[:H] / oT[H : H + 1]).T, dtype=np.float32)


def kernel(x, Wk, Wq, Wv):
    from concourse.bass_utils import run_bass_kernel_spmd

    nc = _get_nc()
    in_maps = make_in_maps(x, Wk, Wq, Wv)
    r = run_bass_kernel_spmd(nc, in_maps, core_ids=list(range(B)))
    out = np.stack([finalize_host(r.results[b]["oT"]) for b in range(B)])
    return np.ascontiguousarray(out, dtype=np.float32)
